# revision 2
# baseline (speedup 1.0000x reference)
"""nn_CausalGATLayer: hybrid Trainium kernel.

Branch 2 (the O(N^2*HID) causal pairwise branch) runs on 8 NeuronCores,
row-sharded over i (64 rows/core). Everything else (O(N*D^2) matmuls,
masked row softmaxes, sort/gather, layernorm) is negligible and runs on host.

Device math per core c (rows i in [64c, 64c+64)):
  M_i[h, j] = relu(rA[i,h] + rB[j,h])            # ACT, bias=per-partition rA col
  s[i, j]   = sum_h w2c[h] * M_i[h, j]           # PE, lhsT=w2c chunk (128,1)
  s[i, i_global] += -1e30                        # per-core dmask input
  E = exp(s)  (global softmax, no max-shift: |s| < ~6 for these inputs)
  RS[i] = sum_j E[i, j]                          # activation accum_out
  G[h] += sum_j E[i, j] * M_i[h, j]              # DVE tensor_tensor_reduce chain
Host: Z = sum_c sum RS_c ; H2vec = (sum_c G_c / Z) @ ce_w2.T + ce_b2
"""

import numpy as np

N, IN, HID, OUT, HD = 512, 256, 256, 256, 64
NC = 8
RPC = N // NC  # rows per core


def _build_device_kernel():
    import concourse.bass as bass
    import concourse.mybir as mybir
    from concourse.tile import TileContext

    f32 = mybir.dt.float32
    nc = bass.Bass()

    rATd = nc.dram_tensor("rAT", [HID, RPC], f32, kind="ExternalInput")
    rBTd = nc.dram_tensor("rBT", [HID, N], f32, kind="ExternalInput")
    w2cd = nc.dram_tensor("w2c", [HID, 1], f32, kind="ExternalInput")
    dmd = nc.dram_tensor("dmask", [RPC, N], f32, kind="ExternalInput")
    Gd = nc.dram_tensor("G", [HID, 1], f32, kind="ExternalOutput")
    RSd = nc.dram_tensor("RS", [RPC, 1], f32, kind="ExternalOutput")

    KC = HID // 128  # 2 contraction chunks of 128 partitions

    with TileContext(nc) as tc:
        with (
            tc.tile_pool(name="const", bufs=1) as cpool,
            tc.tile_pool(name="m", bufs=4) as mpool,
            tc.tile_pool(name="sc", bufs=3) as spool,
            tc.tile_pool(name="ps", bufs=2, space="PSUM") as pspool,
        ):
            rbt = []
            rat = []
            wt = []
            for k in range(KC):
                t = cpool.tile([128, N], f32, tag=f"rbt{k}")
                nc.sync.dma_start(out=t[:, :], in_=rBTd[k * 128:(k + 1) * 128, :])
                rbt.append(t)
                t = cpool.tile([128, RPC], f32, tag=f"rat{k}")
                nc.sync.dma_start(out=t[:, :], in_=rATd[k * 128:(k + 1) * 128, :])
                rat.append(t)
                t = cpool.tile([128, 1], f32, tag=f"wt{k}")
                nc.sync.dma_start(out=t[:, :], in_=w2cd[k * 128:(k + 1) * 128, :])
                wt.append(t)
            dm = cpool.tile([RPC, N], f32, tag="dm")
            nc.sync.dma_start(out=dm[:, :], in_=dmd[:, :])

            S = cpool.tile([RPC, N], f32, tag="S")
            E = cpool.tile([RPC, N], f32, tag="E")
            rs = cpool.tile([RPC, 1], f32, tag="rs")
            G = []
            for k in range(KC):
                g = cpool.tile([128, 1], f32, tag=f"g{k}")
                nc.vector.memset(g[:, :], 0.0)
                G.append(g)

            relu = mybir.ActivationFunctionType.Relu
            expf = mybir.ActivationFunctionType.Exp

            # ---- pass A: scores ----
            for i in range(RPC):
                ps = pspool.tile([1, N], f32, tag="ps")
                for k in range(KC):
                    m = mpool.tile([128, N], f32, tag="m")
                    nc.scalar.activation(m[:, :], rbt[k][:, :], relu,
                                         bias=rat[k][:, i:i + 1])
                    nc.tensor.matmul(ps[0:1, :], wt[k][:, 0:1], m[:, :],
                                     start=(k == 0), stop=(k == KC - 1))
                nc.vector.tensor_copy(S[i:i + 1, :], ps[0:1, :])

            # diagonal mask then global-softmax numerator + row sums
            nc.vector.tensor_add(S[:, :], S[:, :], dm[:, :])
            nc.scalar.activation(E[:, :], S[:, :], expf, accum_out=rs[:, :])

            # ---- pass B: G[h] = sum_ij E_ij * M_i[h, j] ----
            for i in range(RPC):
                for k in range(KC):
                    m = mpool.tile([128, N], f32, tag="m")
                    nc.scalar.activation(m[:, :], rbt[k][:, :], relu,
                                         bias=rat[k][:, i:i + 1])
                    sc = spool.tile([128, N], f32, tag="sc")
                    m_b, e_b = bass.broadcast_tensor_aps(m[:, :], E[i:i + 1, :])
                    nc.vector.tensor_tensor_reduce(
                        out=sc[:, :], in0=m_b, in1=e_b, scale=1.0,
                        scalar=G[k][:, 0:1],
                        op0=mybir.AluOpType.mult, op1=mybir.AluOpType.add,
                        accum_out=G[k][:, 0:1])

            for k in range(KC):
                nc.sync.dma_start(out=Gd[k * 128:(k + 1) * 128, :], in_=G[k][:, :])
            nc.sync.dma_start(out=RSd[:, :], in_=rs[:, :])

    return nc


_NC_CACHE = {}


def _branch2_device(rA, rB, w2c):
    from concourse.bass_utils import run_bass_kernel_spmd

    if "nc" not in _NC_CACHE:
        _NC_CACHE["nc"] = _build_device_kernel()
    nc = _NC_CACHE["nc"]

    rBT = np.ascontiguousarray(rB.T, dtype=np.float32)
    w2cc = np.ascontiguousarray(w2c.reshape(HID, 1), dtype=np.float32)
    in_maps = []
    for c in range(NC):
        rAT = np.ascontiguousarray(rA[c * RPC:(c + 1) * RPC].T, dtype=np.float32)
        dmask = np.zeros((RPC, N), dtype=np.float32)
        for li in range(RPC):
            dmask[li, c * RPC + li] = -1e30
        in_maps.append({"rAT": rAT, "rBT": rBT, "w2c": w2cc, "dmask": dmask})

    res = run_bass_kernel_spmd(nc, in_maps, list(range(NC)))
    globals()["_LAST_RESULT"] = res
    results = res.results
    Z = np.float64(0.0)
    Gtot = np.zeros(HID, dtype=np.float64)
    for r in results:
        Z += np.asarray(r["RS"], dtype=np.float64).sum()
        Gtot += np.asarray(r["G"], dtype=np.float64)[:, 0]
    return (Gtot / Z).astype(np.float32)


def _branch2_host(rA, rB, w2c):
    Z = 0.0
    Gtot = np.zeros(HID, dtype=np.float64)
    for c in range(NC):
        blk = slice(c * RPC, (c + 1) * RPC)
        h = np.maximum(rA[blk][:, None, :] + rB[None, :, :], 0.0)  # (64,512,256)
        s = h @ w2c  # (64, 512)
        for li in range(RPC):
            s[li, c * RPC + li] = -np.inf
        E = np.exp(s)
        Z += E.sum()
        Gtot += np.einsum("ij,ijh->h", E, h, optimize=True)
    return (Gtot / Z).astype(np.float32)


def _softmax_rows(s):
    mx = np.max(s, axis=1, keepdims=True)
    e = np.exp(s - mx)
    return e / e.sum(axis=1, keepdims=True)


def kernel(V, adj, prev_hidden, W1, sa0, sa1, ce_w1, ce_b1, ce_w2, ce_b2, ca0, ca1,
           te_w1, te_b1, te_w2, te_b2, ta0, ta1, pe_w1, pe_b1, pe_w2, pe_b2, pa0, pa1,
           W2, op_w, op_b, ln_g, ln_b):
    V = np.asarray(V, dtype=np.float32)
    adj = np.asarray(adj)
    prev_hidden = np.asarray(prev_hidden, dtype=np.float32)
    fa = lambda x: np.asarray(x, dtype=np.float32)
    (W1, sa0, sa1, ce_w1, ce_b1, ce_w2, ce_b2, ca0, ca1, te_w1, te_b1, te_w2,
     te_b2, ta0, ta1, pe_w1, pe_b1, pe_w2, pe_b2, pa0, pa1, W2, op_w, op_b,
     ln_g, ln_b) = map(fa, (W1, sa0, sa1, ce_w1, ce_b1, ce_w2, ce_b2, ca0, ca1,
                            te_w1, te_b1, te_w2, te_b2, ta0, ta1, pe_w1, pe_b1,
                            pe_w2, pe_b2, pa0, pa1, W2, op_w, op_b, ln_g, ln_b))

    # ---- branch 2 prep (shared by device + host paths) ----
    wA, wB = ce_w1[:, :IN], ce_w1[:, IN:]
    rA = V @ wA.T + ce_b1          # (N, HID), b1 folded in
    rB = V @ wB.T                  # (N, HID)
    c2 = ca0 + ca1                 # (HD,)
    w2c = ce_w2.T @ c2             # (HID,)

    Gn = None
    try:
        import signal

        class _TO(Exception):
            pass

        def _h(s, f):
            raise _TO()

        timer_set = False
        try:
            signal.signal(signal.SIGALRM, _h)
            signal.alarm(1200)
            timer_set = True
        except Exception:
            pass
        try:
            Gn = _branch2_device(rA, rB, w2c)
        finally:
            if timer_set:
                signal.alarm(0)
        if not np.all(np.isfinite(Gn)):
            Gn = None
    except Exception:
        Gn = None
    if Gn is None:
        Gn = _branch2_host(rA, rB, w2c)

    H2v = Gn @ ce_w2.T + ce_b2     # (HD,)
    H2 = np.broadcast_to(H2v, (N, HD))

    # ---- branch 1: standard GAT ----
    Wh1 = V @ W1.T
    s1 = (Wh1 @ sa0)[:, None] + (Wh1 @ sa1)[None, :]
    s1 = np.where(adj == 0, -np.inf, s1)
    H1 = _softmax_rows(s1) @ Wh1

    # ---- branch 3: temporal prefix means ----
    x3 = np.concatenate([V, prev_hidden], axis=-1)
    tf = np.maximum(x3 @ te_w1.T + te_b1, 0.0) @ te_w2.T + te_b2  # (N, HD)
    H3 = np.cumsum(tf, axis=0) / np.arange(1, N + 1, dtype=np.float32)[:, None]

    # ---- branch 4: first two neighbors ----
    ar = np.arange(N)
    pos = np.where(adj == 1, ar[None, :], N)
    srt = np.sort(pos, axis=1)
    i0, i1 = srt[:, 0], srt[:, 1]
    valid = (i1 < N)[:, None]
    n0 = np.where(valid, V[np.clip(i0, 0, N - 1)], 0.0)
    n1 = np.where(valid, V[np.clip(i1, 0, N - 1)], 0.0)
    x4 = np.concatenate([V, n0, n1], axis=-1)
    cf = np.maximum(x4 @ pe_w1.T + pe_b1, 0.0) @ pe_w2.T + pe_b2  # (N, HD)
    H4v = cf.sum(axis=0)
    H4 = np.concatenate([H4v, np.zeros(N - HD, dtype=np.float32)])[:, None]

    # ---- combine ----
    Hc = np.concatenate([H1, H2, H3, H4], axis=-1) @ W2.T
    out = Hc @ op_w.T + op_b
    mu = out.mean(-1, keepdims=True)
    var = ((out - mu) ** 2).mean(-1, keepdims=True)
    y = (out - mu) / np.sqrt(var + 1e-5) * ln_g + ln_b
    return np.where(y > 0, y, np.expm1(y)).astype(np.float32)



# revision 11
# speedup vs baseline: 1.4684x; 1.4684x over previous
"""nn_CausalGATLayer: hybrid Trainium kernel.

Branch 2 (the O(N^2*HID) causal pairwise branch) runs on 8 NeuronCores,
row-sharded over i (64 rows/core). Everything else (O(N*D^2) matmuls,
masked row softmaxes, sort/gather, layernorm) is negligible and runs on host.

Device math per core c (rows i in [64c, 64c+64)), all pairwise tiles bf16:
  M_i[h, j] = relu(rA[i,h] + rB[j,h])        # DVE tensor_scalar (add+max) / ACT
  s[i, j]   = sum_h w2c[h] * M_i[h, j]       # PE, lhsT=w2c chunk (128,1), psum
  E = exp(s), RS[i] = sum_j E[i, j]          # ACT batched exp + accum_out
  E -> DRAM scratch -> per-row broadcast DMA to [128, 512] tiles
  G[h] += sum_j M_i[h, j] * E[i, j]          # DVE TTR / GPSIMD STT (split)
Host: subtract exact diagonal terms, Z = sum RS - sum_diag,
      H2vec = (G/Z) @ ce_w2.T + ce_b2.
"""

import numpy as np

N, IN, HID, OUT, HD = 512, 256, 256, 256, 64
NC = 8
RPC = N // NC  # rows per core

# chunk scheduling: c = 2*i + k indexes the 128 [128,512] chunks per core.
# production: ACT if (c % 16) < _ACT_PROD else DVE
# G-pass:     DVE if (c % 16) < _DVE_G else GPSIMD
_ACT_PROD = 5
_DVE_G = 7


def _build_device_kernel():
    import concourse.bass as bass
    import concourse.mybir as mybir
    from concourse.tile import TileContext

    f32 = mybir.dt.float32
    bf16 = mybir.dt.bfloat16
    A = mybir.AluOpType
    relu = mybir.ActivationFunctionType.Relu
    expf = mybir.ActivationFunctionType.Exp

    nc = bass.Bass()

    rbtd = nc.dram_tensor("rbt", [HID, N], bf16, kind="ExternalInput")
    ratd = nc.dram_tensor("rat", [HID, RPC], f32, kind="ExternalInput")
    # w2c chunk replicated to 128 columns (stationary for the s matmuls, so
    # psum rows 0..127 all receive s_i -- a free partition-broadcast)
    w2crd = nc.dram_tensor("w2cr", [HID, 128], bf16, kind="ExternalInput")
    gps_per_k = [sum(1 for i in range(RPC) if ((2 * i + k) % 16) >= _DVE_G)
                 for k in range(2)]
    gps_cols = max(gps_per_k)
    Gd = nc.dram_tensor("G", [HID, 1 + gps_cols], f32, kind="ExternalOutput")
    Zd = nc.dram_tensor("Zp", [1, RPC // 4], f32, kind="ExternalOutput")

    KC = HID // 128  # 2 contraction chunks of 128 partitions
    NQ = RPC // 4    # 16 quads of 4 rows

    with TileContext(nc) as tc:
        with (
            tc.tile_pool(name="const", bufs=1) as cpool,
            tc.tile_pool(name="eb", bufs=3) as ebpool,
            tc.tile_pool(name="ps", bufs=2, space="PSUM") as pspool,
        ):
            rbt, rat, w2cr, M = [], [], [], []
            for k in range(KC):
                t = cpool.tile([128, N], bf16, tag=f"rbt{k}")
                nc.sync.dma_start(out=t[:, :], in_=rbtd[k * 128:(k + 1) * 128, :])
                rbt.append(t)
                t = cpool.tile([128, RPC], f32, tag=f"rat{k}")
                nc.sync.dma_start(out=t[:, :], in_=ratd[k * 128:(k + 1) * 128, :])
                rat.append(t)
                t = cpool.tile([128, 128], bf16, tag=f"w2cr{k}")
                nc.sync.dma_start(out=t[:, :], in_=w2crd[k * 128:(k + 1) * 128, :])
                w2cr.append(t)
                mt = cpool.tile([128, RPC * N], bf16, tag=f"M{k}")
                M.append(mt)

            gd, gg = [], []
            for k in range(KC):
                g = cpool.tile([128, 1], f32, tag=f"gd{k}")
                nc.vector.memset(g[:, :], 0.0)
                gd.append(g)
                g = cpool.tile([128, gps_cols], f32, tag=f"gg{k}")
                nc.vector.memset(g[:, :], 0.0)
                gg.append(g)

            zall = cpool.tile([128, NQ], f32, tag="zall")
            sc_d = cpool.tile([128, N], bf16, tag="sc_d")
            sc_g = cpool.tile([128, N], bf16, tag="sc_g")

            gcount = [0, 0]
            for q in range(NQ):
                sb4 = pspool.tile([128, 4 * N], f32, tag="sb4")
                for ii in range(4):
                    i = q * 4 + ii
                    for k in range(KC):
                        c = 2 * i + k
                        dst = M[k][:, i * N:(i + 1) * N]
                        if (c % 16) < _ACT_PROD:
                            nc.scalar.activation(dst, rbt[k][:, :], relu,
                                                 bias=rat[k][:, i:i + 1])
                        else:
                            nc.vector.tensor_scalar(
                                out=dst, in0=rbt[k][:, :],
                                scalar1=rat[k][:, i:i + 1], scalar2=0.0,
                                op0=A.add, op1=A.max)
                        nc.tensor.matmul(sb4[:, ii * N:(ii + 1) * N],
                                         w2cr[k][:, :], dst,
                                         start=(k == 0), stop=(k == KC - 1))
                eb4 = ebpool.tile([128, 4 * N], bf16, tag="eb4")
                nc.scalar.activation(eb4[:, :], sb4[:, :], expf,
                                     accum_out=zall[:, q:q + 1])
                for ii in range(4):
                    i = q * 4 + ii
                    ebsl = eb4[:, ii * N:(ii + 1) * N]
                    for k in range(KC):
                        c = 2 * i + k
                        msl = M[k][:, i * N:(i + 1) * N]
                        if (c % 16) < _DVE_G:
                            nc.vector.tensor_tensor_reduce(
                                out=sc_d[:, :], in0=msl, in1=ebsl, scale=1.0,
                                scalar=gd[k][:, 0:1], op0=A.mult, op1=A.add,
                                accum_out=gd[k][:, 0:1])
                        else:
                            col = gcount[k]
                            gcount[k] += 1
                            nc.gpsimd.scalar_tensor_tensor(
                                out=sc_g[:, :], in0=msl, scalar=0.0, in1=ebsl,
                                op0=A.bypass, op1=A.mult,
                                accum_out=gg[k][:, col:col + 1])

            for k in range(KC):
                nc.sync.dma_start(out=Gd[k * 128:(k + 1) * 128, 0:1], in_=gd[k][:, :])
                nc.sync.dma_start(out=Gd[k * 128:(k + 1) * 128, 1:1 + gps_cols],
                                  in_=gg[k][:, :])
            nc.sync.dma_start(out=Zd[0:1, :], in_=zall[0:1, :])

    return nc, gps_cols


_NC_CACHE = {}


def _branch2_device(rA, rB, w2c):
    import ml_dtypes
    from concourse.bass_utils import run_bass_kernel_spmd

    if "nc" not in _NC_CACHE:
        _NC_CACHE["nc"] = _build_device_kernel()
    nc, gps_cols = _NC_CACHE["nc"]

    bf16 = ml_dtypes.bfloat16
    rBT = np.ascontiguousarray(rB.T.astype(bf16))                  # [HID, N]
    w2cr = np.ascontiguousarray(
        np.broadcast_to(w2c.reshape(HID, 1), (HID, 128)).astype(bf16))
    in_maps = []
    for c in range(NC):
        rAT = np.ascontiguousarray(rA[c * RPC:(c + 1) * RPC].T, dtype=np.float32)
        in_maps.append({"rbt": rBT, "rat": rAT, "w2cr": w2cr})

    res = run_bass_kernel_spmd(nc, in_maps, list(range(NC)))
    globals()["_LAST_RESULT"] = res

    # col 0 = DVE-chained G; cols 1.. = per-GPS-chunk partial sums (zeroed
    # on device, so summing all columns is safe)
    Z = np.float64(0.0)
    Gtot = np.zeros(HID, dtype=np.float64)
    for r in res.results:
        Z += np.asarray(r["Zp"], dtype=np.float64).sum()
        Gm = np.asarray(r["G"], dtype=np.float64)  # [HID, 1+gps_cols]
        Gtot += Gm.sum(axis=1)

    # exact diagonal correction on host (device summed over ALL j incl. i==j)
    Mdiag = np.maximum(rA + rB, 0.0).astype(np.float64)            # [N, HID]
    s_diag = Mdiag @ w2c.astype(np.float64)                        # [N]
    e_diag = np.exp(s_diag)
    Z -= e_diag.sum()
    Gtot -= e_diag @ Mdiag
    return (Gtot / Z).astype(np.float32)


def _branch2_host(rA, rB, w2c):
    Z = 0.0
    Gtot = np.zeros(HID, dtype=np.float64)
    for c in range(NC):
        blk = slice(c * RPC, (c + 1) * RPC)
        h = np.maximum(rA[blk][:, None, :] + rB[None, :, :], 0.0)  # (64,512,256)
        s = h @ w2c  # (64, 512)
        for li in range(RPC):
            s[li, c * RPC + li] = -np.inf
        E = np.exp(s)
        Z += E.sum()
        Gtot += np.einsum("ij,ijh->h", E, h, optimize=True)
    return (Gtot / Z).astype(np.float32)


def _softmax_rows(s):
    mx = np.max(s, axis=1, keepdims=True)
    e = np.exp(s - mx)
    return e / e.sum(axis=1, keepdims=True)


def kernel(V, adj, prev_hidden, W1, sa0, sa1, ce_w1, ce_b1, ce_w2, ce_b2, ca0, ca1,
           te_w1, te_b1, te_w2, te_b2, ta0, ta1, pe_w1, pe_b1, pe_w2, pe_b2, pa0, pa1,
           W2, op_w, op_b, ln_g, ln_b):
    V = np.asarray(V, dtype=np.float32)
    adj = np.asarray(adj)
    prev_hidden = np.asarray(prev_hidden, dtype=np.float32)
    fa = lambda x: np.asarray(x, dtype=np.float32)
    (W1, sa0, sa1, ce_w1, ce_b1, ce_w2, ce_b2, ca0, ca1, te_w1, te_b1, te_w2,
     te_b2, ta0, ta1, pe_w1, pe_b1, pe_w2, pe_b2, pa0, pa1, W2, op_w, op_b,
     ln_g, ln_b) = map(fa, (W1, sa0, sa1, ce_w1, ce_b1, ce_w2, ce_b2, ca0, ca1,
                            te_w1, te_b1, te_w2, te_b2, ta0, ta1, pe_w1, pe_b1,
                            pe_w2, pe_b2, pa0, pa1, W2, op_w, op_b, ln_g, ln_b))

    # ---- branch 2 prep (shared by device + host paths) ----
    wA, wB = ce_w1[:, :IN], ce_w1[:, IN:]
    rA = V @ wA.T + ce_b1          # (N, HID), b1 folded in
    rB = V @ wB.T                  # (N, HID)
    c2 = ca0 + ca1                 # (HD,)
    w2c = ce_w2.T @ c2             # (HID,)

    Gn = None
    try:
        import signal

        class _TO(Exception):
            pass

        def _h(s, f):
            raise _TO()

        timer_set = False
        try:
            signal.signal(signal.SIGALRM, _h)
            signal.alarm(1200)
            timer_set = True
        except Exception:
            pass
        try:
            Gn = _branch2_device(rA, rB, w2c)
        finally:
            if timer_set:
                signal.alarm(0)
        if not np.all(np.isfinite(Gn)):
            Gn = None
    except Exception:
        Gn = None
    if Gn is None:
        Gn = _branch2_host(rA, rB, w2c)

    H2v = Gn @ ce_w2.T + ce_b2     # (HD,)
    H2 = np.broadcast_to(H2v, (N, HD))

    # ---- branch 1: standard GAT ----
    Wh1 = V @ W1.T
    s1 = (Wh1 @ sa0)[:, None] + (Wh1 @ sa1)[None, :]
    s1 = np.where(adj == 0, -np.inf, s1)
    H1 = _softmax_rows(s1) @ Wh1

    # ---- branch 3: temporal prefix means ----
    x3 = np.concatenate([V, prev_hidden], axis=-1)
    tf = np.maximum(x3 @ te_w1.T + te_b1, 0.0) @ te_w2.T + te_b2  # (N, HD)
    H3 = np.cumsum(tf, axis=0) / np.arange(1, N + 1, dtype=np.float32)[:, None]

    # ---- branch 4: first two neighbors ----
    ar = np.arange(N)
    pos = np.where(adj == 1, ar[None, :], N)
    srt = np.sort(pos, axis=1)
    i0, i1 = srt[:, 0], srt[:, 1]
    valid = (i1 < N)[:, None]
    n0 = np.where(valid, V[np.clip(i0, 0, N - 1)], 0.0)
    n1 = np.where(valid, V[np.clip(i1, 0, N - 1)], 0.0)
    x4 = np.concatenate([V, n0, n1], axis=-1)
    cf = np.maximum(x4 @ pe_w1.T + pe_b1, 0.0) @ pe_w2.T + pe_b2  # (N, HD)
    H4v = cf.sum(axis=0)
    H4 = np.concatenate([H4v, np.zeros(N - HD, dtype=np.float32)])[:, None]

    # ---- combine ----
    Hc = np.concatenate([H1, H2, H3, H4], axis=-1) @ W2.T
    out = Hc @ op_w.T + op_b
    mu = out.mean(-1, keepdims=True)
    var = ((out - mu) ** 2).mean(-1, keepdims=True)
    y = (out - mu) / np.sqrt(var + 1e-5) * ln_g + ln_b
    return np.where(y > 0, y, np.expm1(y)).astype(np.float32)
